# revision 13
# baseline (speedup 1.0000x reference)
"""Trainium2 Bass kernel for the GNN ExplainModule (masked adjacency).

Strategy (8 NeuronCores, row-sharded output):
  - Each core owns 1250 rows of the [10000, 10000] output, processed in
    row-blocks of 128.
  - Host routes each edge's two contributions ((r,c) and (c,r), weight
    0.5*gate) to the owning core/block, sorted by destination; indices
    only — all FP math runs on device.
  - Device tables via PE: A = (embed @ W1a + 1 x c_vec) * |W2|,
    B = (embed @ W1b) * |W2|  (hidden units permuted so W2 >= 0 first;
    signs re-applied as pos-reduce minus neg-reduce).
  - Per contribution: dma_gather A[row], B[col] and the 64-wide adj
    segment holding (r, c); compute gate = sigmoid(logit(noise) + mlp);
    payload = onehot64(c % 64) * adj_seg * (0.5 * gate * valid);
    dma_scatter_add payload into the output (CCE add; duplicate dests
    accumulate natively; output buffers arrive pre-zeroed via PJRT
    donation so untouched cells stay 0).
"""

import sys

import numpy as np

for _p in ("/opt/trn_rl_repo",):
    if _p not in sys.path:
        sys.path.insert(0, _p)

N = 10000
D = 64
NCORES = 8
RPC = N // NCORES  # rows per core
BLK = 128  # rows per block
SEG = -(-N // 64)  # 64-wide segments per row (157)
SEGX = SEG + 1  # +1 pad segment per row (scatter pad target)
PITCH = SEGX * 64  # padded row pitch
SUB = 256  # tokens per custom-DMA op (empirically safe on HW)


def _blocks():
    out = []
    r = 0
    while r < RPC:
        h = min(BLK, RPC - r)
        out.append((r, h))
        r += h
    return out


def _prep_host(row, col, noise):
    """Route contributions to (core, block); build packed token arrays."""
    row = np.asarray(row).astype(np.int64).ravel()
    col = np.asarray(col).astype(np.int64).ravel()
    noise = np.asarray(noise).astype(np.float32).ravel()

    dr = np.concatenate([row, col])  # dest row
    dc = np.concatenate([col, row])  # dest col
    ea = np.concatenate([row, row])  # A-table index
    eb = np.concatenate([col, col])  # B-table index
    en = np.concatenate([noise, noise])
    core = dr // RPC

    blocks = _blocks()
    nblk = len(blocks)
    # per core, per block, per wave: token arrays. A scatter instruction must
    # not carry two tokens targeting the same 64-wide segment row (the HW CCE
    # adds race within one instruction); the w-th token of each segment group
    # goes to wave w, and waves scatter in separate, serialized instructions.
    toks = [[None] * nblk for _ in range(NCORES)]
    n_waves = 1
    for k in range(NCORES):
        m = core == k
        rl = dr[m] - k * RPC
        d = rl * N + dc[m]
        o = np.argsort(d, kind="stable")
        rl, dcc, a, b, nz = rl[o], dc[m][o], ea[m][o], eb[m][o], en[m][o]
        blk_id = rl // BLK
        for bi, (r0, h) in enumerate(blocks):
            sel = blk_id == bi
            si = (rl[sel] - r0) * SEGX + dcc[sel] // 64
            # occurrence rank of each token within its segment group (tokens
            # are sorted by dest, so equal si values are adjacent)
            uq, inv, cnt = np.unique(si, return_inverse=True, return_counts=True)
            starts = np.zeros(len(uq) + 1, np.int64)
            np.cumsum(cnt, out=starts[1:])
            rank = np.arange(len(si)) - starts[inv]
            n_waves = max(n_waves, int(cnt.max()) if len(cnt) else 1)
            toks[k][bi] = (
                a[sel],
                b[sel],
                nz[sel],
                si,
                (dcc[sel] % 64).astype(np.float32),
                rank,
            )

    # SPMD-static chunk sizes per (block, wave)
    chunk_list = []  # (block_idx, row0, blk_h, t, off16, off128)
    key_sizes = {}  # (bi, w) -> padded size
    off16 = off128 = 0
    for bi, (r0, h) in enumerate(blocks):
        for w in range(n_waves):
            t_bw = max(
                int((toks[k][bi][5] == w).sum()) for k in range(NCORES)
            )
            if w == 0:
                t_bw = max(t_bw, 1)
            if t_bw == 0:
                continue
            t_bw = -(-t_bw // 128) * 128
            key_sizes[(bi, w)] = t_bw
            done = 0
            while done < t_bw:
                t = min(SUB, t_bw - done)
                chunk_list.append((bi, r0, h, t, off16, off128))
                off16 += t // 16
                off128 += t // 128
                done += t
    total16, total128 = off16, off128

    pad_si = SEGX - 1  # row 0's pad segment; never holds real data

    per_core = []
    for k in range(NCORES):
        ga16 = np.zeros((128, total16), np.int16)
        gb16 = np.zeros((128, total16), np.int16)
        si16 = np.full((128, total16), 0, np.int16)
        nzf = np.full((128, total128), 0.5, np.float32)
        cmf = np.zeros((128, total128), np.float32)
        vmf = np.zeros((128, total128), np.float32)
        ci = 0
        for bi, (r0, h) in enumerate(blocks):
            a0, b0, nz0, si0, cm0, rank0 = toks[k][bi]
            for w in range(n_waves):
                if (bi, w) not in key_sizes:
                    continue
                t_bw = key_sizes[(bi, w)]
                sel = rank0 == w
                n = int(sel.sum())
                pad = t_bw - n
                a = np.concatenate([a0[sel], np.zeros(pad, np.int64)])
                b = np.concatenate([b0[sel], np.zeros(pad, np.int64)])
                nz = np.concatenate([nz0[sel], np.full(pad, 0.5, np.float32)])
                si = np.concatenate([si0[sel], np.full(pad, pad_si, np.int64)])
                cm = np.concatenate([cm0[sel], np.zeros(pad, np.float32)])
                vm = np.concatenate(
                    [np.ones(n, np.float32), np.zeros(pad, np.float32)]
                )
                done = 0
                while done < t_bw:
                    bi2, _r0, _h, t, o16, o128 = chunk_list[ci]
                    assert bi2 == bi and done + t <= t_bw
                    sl = slice(done, done + t)

                    def wrap16(x):
                        return np.tile(
                            np.ascontiguousarray(x[sl].reshape(-1, 16).T),
                            (8, 1),
                        )

                    def wrap128(x):
                        return np.ascontiguousarray(x[sl].reshape(-1, 128).T)

                    ga16[:, o16 : o16 + t // 16] = wrap16(a).astype(np.int16)
                    gb16[:, o16 : o16 + t // 16] = wrap16(b).astype(np.int16)
                    si16[:, o16 : o16 + t // 16] = wrap16(si).astype(np.int16)
                    nzf[:, o128 : o128 + t // 128] = wrap128(nz)
                    cmf[:, o128 : o128 + t // 128] = wrap128(cm)
                    vmf[:, o128 : o128 + t // 128] = wrap128(vm)
                    done += t
                    ci += 1
        assert ci == len(chunk_list)
        per_core.append(
            dict(ga16=ga16, gb16=gb16, si16=si16, nz=nzf, cm=cmf, vm=vmf)
        )
    return per_core, chunk_list, total16, total128


def _build_program(chunk_list, total16, total128, node_idx, b2f, pos_cnt):
    import concourse.bacc as bacc
    import concourse.bass as bass
    import concourse.mybir as mybir
    import concourse.tile as tile
    from concourse.masks import make_identity

    f32 = mybir.dt.float32
    i16 = mybir.dt.int16
    add = mybir.AluOpType.add
    mult = mybir.AluOpType.mult
    subtract = mybir.AluOpType.subtract
    is_equal = mybir.AluOpType.is_equal
    AF = mybir.ActivationFunctionType

    nc = bacc.Bacc()

    blocks = _blocks()
    out_rows = sum(BLK for _ in blocks)  # padded block heights (128 each)

    embp = nc.declare_dram_parameter("embed", [N, D], f32, isOutput=False)
    w1p = nc.declare_dram_parameter("w1", [3 * D, D], f32, isOutput=False)
    b1p = nc.declare_dram_parameter("b1r", [1, D], f32, isOutput=False)
    w2p = nc.declare_dram_parameter("w2b", [128, D], f32, isOutput=False)
    iop = nc.declare_dram_parameter("iota64", [128, D], f32, isOutput=False)
    adjp = nc.declare_dram_parameter("adjp", [out_rows, PITCH], f32, isOutput=False)
    gap = nc.declare_dram_parameter("ga16", [128, total16], i16, isOutput=False)
    gbp = nc.declare_dram_parameter("gb16", [128, total16], i16, isOutput=False)
    sip = nc.declare_dram_parameter("si16", [128, total16], i16, isOutput=False)
    nzp = nc.declare_dram_parameter("nz", [128, total128], f32, isOutput=False)
    cmp_ = nc.declare_dram_parameter("cm", [128, total128], f32, isOutput=False)
    vmp = nc.declare_dram_parameter("vm", [128, total128], f32, isOutput=False)
    outp = nc.declare_dram_parameter("out", [out_rows, PITCH], f32, isOutput=True)

    a_dram = nc.dram_tensor("a_table", [N, D], f32)
    b_dram = nc.dram_tensor("b_table", [N, D], f32)

    NBLKA = -(-N // 128)

    with tile.TileContext(nc) as tc:
        with (
            tc.tile_pool(name="const", bufs=1) as cp,
            tc.tile_pool(name="stagea", bufs=3) as sp,
            tc.tile_pool(name="work", bufs=2) as wp,
            tc.tile_pool(name="psum", bufs=2, space="PSUM") as pp,
        ):
            identity = cp.tile([128, 128], f32)
            make_identity(nc, identity[:])
            w1a = cp.tile([D, D], f32)
            nc.sync.dma_start(out=w1a[:], in_=w1p[0:D, :])
            w1b = cp.tile([D, D], f32)
            nc.sync.dma_start(out=w1b[:], in_=w1p[D : 2 * D, :])
            w1c = cp.tile([D, D], f32)
            nc.sync.dma_start(out=w1c[:], in_=w1p[2 * D : 3 * D, :])
            b1t = cp.tile([1, D], f32)
            nc.sync.dma_start(out=b1t[:], in_=b1p[:, :])
            w2t = cp.tile([128, D], f32)
            nc.sync.dma_start(out=w2t[:], in_=w2p[:, :])
            iot = cp.tile([128, D], f32)
            nc.sync.dma_start(out=iot[:], in_=iop[:, :])
            ones = cp.tile([1, 128], f32)
            nc.vector.memset(ones[:], 1.0)
            e5 = cp.tile([D, 1], f32)
            nc.sync.dma_start(
                out=e5[:], in_=embp[node_idx : node_idx + 1, :].rearrange("o d -> d o")
            )

            # c_vec = embed[node_idx] @ W1c + b1  -> [1, D]
            cps = pp.tile([1, D], f32, tag="cps")
            nc.tensor.matmul(cps[:], lhsT=e5[:], rhs=w1c[:], start=True, stop=True)
            crow = cp.tile([1, D], f32)
            nc.vector.tensor_tensor(out=crow[:], in0=cps[:], in1=b1t[:], op=add)

            # Stage A: A = (embed @ W1a + 1 x crow) * |W2| ; B = (embed @ W1b) * |W2|
            for blk in range(NBLKA):
                r0 = blk * 128
                p = min(128, N - r0)
                et = sp.tile([128, D], f32, tag="et")
                nc.sync.dma_start(out=et[:p, :], in_=embp[r0 : r0 + p, :])
                tps = pp.tile([D, 128], f32, tag="tps")
                nc.tensor.transpose(tps[:, :p], et[:p, :], identity[:p, :p])
                tsb = sp.tile([D, 128], f32, tag="tsb")
                nc.scalar.copy(out=tsb[:, :p], in_=tps[:, :p])
                pa_ = pp.tile([128, D], f32, tag="pa")
                nc.tensor.matmul(
                    pa_[:p, :], lhsT=tsb[:, :p], rhs=w1a[:], start=True, stop=False
                )
                nc.tensor.matmul(
                    pa_[:p, :], lhsT=ones[:, :p], rhs=crow[:], start=False, stop=True
                )
                asb = sp.tile([128, D], f32, tag="asb")
                nc.vector.tensor_tensor(
                    out=asb[:p, :], in0=pa_[:p, :], in1=w2t[:p, :], op=mult
                )
                nc.sync.dma_start(out=a_dram[r0 : r0 + p, :], in_=asb[:p, :])
                pb_ = pp.tile([128, D], f32, tag="pb")
                nc.tensor.matmul(
                    pb_[:p, :], lhsT=tsb[:, :p], rhs=w1b[:], start=True, stop=True
                )
                bsb = sp.tile([128, D], f32, tag="bsb")
                nc.vector.tensor_tensor(
                    out=bsb[:p, :], in0=pb_[:p, :], in1=w2t[:p, :], op=mult
                )
                nc.sync.dma_start(out=b_dram[r0 : r0 + p, :], in_=bsb[:p, :])

            # contribution chunks
            for bi, r0b, h, t, o16, o128 in chunk_list:
                S = t // 128
                S16 = t // 16
                gai = wp.tile([128, S16], i16, tag="gai")
                nc.sync.dma_start(out=gai[:], in_=gap[:, o16 : o16 + S16])
                gbi = wp.tile([128, S16], i16, tag="gbi")
                nc.sync.dma_start(out=gbi[:], in_=gbp[:, o16 : o16 + S16])
                sii = wp.tile([128, S16], i16, tag="sii")
                nc.sync.dma_start(out=sii[:], in_=sip[:, o16 : o16 + S16])
                nz = wp.tile([128, S], f32, tag="nz")
                nc.sync.dma_start(out=nz[:], in_=nzp[:, o128 : o128 + S])
                cm = wp.tile([128, S], f32, tag="cm")
                nc.sync.dma_start(out=cm[:], in_=cmp_[:, o128 : o128 + S])
                vm = wp.tile([128, S], f32, tag="vm")
                nc.sync.dma_start(out=vm[:], in_=vmp[:, o128 : o128 + S])

                ga = wp.tile([128, S * D], f32, tag="ga")
                nc.gpsimd.dma_gather(
                    out_ap=ga[:].rearrange("p (s d) -> p s d", d=D),
                    in_ap=a_dram[:, :],
                    idxs_ap=gai[:],
                    num_idxs=t,
                    num_idxs_reg=t,
                    elem_size=D,
                )
                gb = wp.tile([128, S * D], f32, tag="gb")
                nc.gpsimd.dma_gather(
                    out_ap=gb[:].rearrange("p (s d) -> p s d", d=D),
                    in_ap=b_dram[:, :],
                    idxs_ap=gbi[:],
                    num_idxs=t,
                    num_idxs_reg=t,
                    elem_size=D,
                )
                adjseg = wp.tile([128, S * D], f32, tag="adjseg")
                adj_view = adjp[r0b : r0b + BLK, :].rearrange(
                    "p (s w) -> (p s) w", w=64
                )
                nc.gpsimd.dma_gather(
                    out_ap=adjseg[:].rearrange("p (s d) -> p s d", d=D),
                    in_ap=adj_view,
                    idxs_ap=sii[:],
                    num_idxs=t,
                    num_idxs_reg=t,
                    elem_size=D,
                )

                # MLP: pre = ga + gb ; q = relu(pre) ; s = sum_pos - sum_neg
                nc.vector.tensor_tensor(out=ga[:], in0=ga[:], in1=gb[:], op=add)
                nc.scalar.activation(out=ga[:], in_=ga[:], func=AF.Relu)
                q3 = ga[:].rearrange("p (s d) -> p s d", d=D)
                s = wp.tile([128, S], f32, tag="s")
                if pos_cnt == D:
                    nc.vector.tensor_reduce(
                        out=s[:], in_=q3, axis=mybir.AxisListType.X, op=add
                    )
                elif pos_cnt == 0:
                    nc.vector.tensor_reduce(
                        out=s[:], in_=q3, axis=mybir.AxisListType.X, op=add,
                        negate=True,
                    )
                else:
                    nc.vector.tensor_reduce(
                        out=s[:], in_=q3[:, :, :pos_cnt],
                        axis=mybir.AxisListType.X, op=add,
                    )
                    sn = wp.tile([128, S], f32, tag="sn")
                    nc.vector.tensor_reduce(
                        out=sn[:], in_=q3[:, :, pos_cnt:],
                        axis=mybir.AxisListType.X, op=add,
                    )
                    nc.vector.tensor_tensor(
                        out=s[:], in0=s[:], in1=sn[:], op=subtract
                    )

                # gate = sigmoid(ln(nz) - ln(1-nz) + s + b2)
                om = wp.tile([128, S], f32, tag="om")
                nc.vector.tensor_scalar(
                    out=om[:], in0=nz[:], scalar1=-1.0, scalar2=1.0,
                    op0=mult, op1=add,
                )
                ln1 = wp.tile([128, S], f32, tag="ln1")
                nc.scalar.activation(out=ln1[:], in_=nz[:], func=AF.Ln)
                ln2 = wp.tile([128, S], f32, tag="ln2")
                nc.scalar.activation(out=ln2[:], in_=om[:], func=AF.Ln)
                z = wp.tile([128, S], f32, tag="z")
                nc.vector.scalar_tensor_tensor(
                    out=z[:], in0=ln1[:], scalar=b2f, in1=ln2[:],
                    op0=add, op1=subtract,
                )
                nc.vector.tensor_tensor(out=z[:], in0=z[:], in1=s[:], op=add)
                g = wp.tile([128, S], f32, tag="g")
                nc.scalar.activation(out=g[:], in_=z[:], func=AF.Sigmoid)
                gm = wp.tile([128, S], f32, tag="gm")
                nc.vector.scalar_tensor_tensor(
                    out=gm[:], in0=g[:], scalar=0.5, in1=vm[:],
                    op0=mult, op1=mult,
                )

                # payload = onehot(cm) * adjseg * gm
                oh = wp.tile([128, S * D], f32, tag="oh")
                oh3 = oh[:].rearrange("p (s d) -> p s d", d=D)
                io_b = iot[:].rearrange("p (o d) -> p o d", o=1).to_broadcast(
                    [128, S, D]
                )
                cm_b = cm[:].rearrange("p (s o) -> p s o", o=1).to_broadcast(
                    [128, S, D]
                )
                nc.vector.tensor_tensor(out=oh3, in0=io_b, in1=cm_b, op=is_equal)
                nc.vector.tensor_tensor(out=oh[:], in0=oh[:], in1=adjseg[:], op=mult)
                gm_b = gm[:].rearrange("p (s o) -> p s o", o=1).to_broadcast(
                    [128, S, D]
                )
                nc.vector.tensor_tensor(out=oh3, in0=oh3, in1=gm_b, op=mult)

                out_view = outp[r0b : r0b + BLK, :].rearrange(
                    "p (s w) -> (p s) w", w=64
                )
                nc.gpsimd.dma_scatter_add(
                    out_ap=out_view,
                    in_ap=oh[:].rearrange("p (s d) -> p s d", d=D),
                    idxs_ap=sii[:],
                    num_idxs=t,
                    num_idxs_reg=t,
                    elem_size=D,
                )

    nc.compile()
    return nc


def kernel(embed, row, col, adj, noise, W1, b1, W2, b2, node_idx):
    from concourse.bass_utils import run_bass_kernel_spmd

    embed = np.ascontiguousarray(np.asarray(embed), dtype=np.float32)
    adj = np.ascontiguousarray(np.asarray(adj), dtype=np.float32)
    W1 = np.ascontiguousarray(np.asarray(W1), dtype=np.float32)
    b1 = np.ascontiguousarray(np.asarray(b1), dtype=np.float32).ravel()
    W2 = np.ascontiguousarray(np.asarray(W2), dtype=np.float32)
    b2f = float(np.asarray(b2, dtype=np.float32).ravel()[0])
    nidx = int(np.asarray(node_idx))

    # permute hidden units: W2 >= 0 first; fold |W2| on device
    w2v = W2.reshape(-1).astype(np.float32)
    order = np.argsort(w2v < 0, kind="stable")
    pos_cnt = int((w2v >= 0).sum())
    W1p = np.ascontiguousarray(W1[:, order])
    b1p = np.ascontiguousarray(b1[order]).reshape(1, D)
    w2b = np.ascontiguousarray(
        np.tile(np.abs(w2v[order]).reshape(1, D), (128, 1))
    )
    iota64 = np.ascontiguousarray(
        np.tile(np.arange(D, dtype=np.float32).reshape(1, D), (128, 1))
    )

    per_core, chunk_list, total16, total128 = _prep_host(row, col, noise)
    nc = _build_program(chunk_list, total16, total128, nidx, b2f, pos_cnt)

    blocks = _blocks()
    out_rows = BLK * len(blocks)
    in_maps = []
    for k in range(NCORES):
        adjpad = np.zeros((out_rows, PITCH), np.float32)
        sl = adj[k * RPC : (k + 1) * RPC]
        adjpad[: sl.shape[0], :N] = sl
        m = dict(per_core[k])
        m.update(
            embed=embed, w1=W1p, b1r=b1p, w2b=w2b, iota64=iota64, adjp=adjpad
        )
        in_maps.append(m)

    res = run_bass_kernel_spmd(nc, in_maps, list(range(NCORES)))
    kernel.last_exec_time_ns = res.exec_time_ns
    pieces = []
    for k in range(NCORES):
        o = res.results[k]["out"]
        # blocks are stacked at BLK spacing; real rows of block bi: r0..r0+h
        for bi, (r0, h) in enumerate(blocks):
            pieces.append(o[bi * BLK : bi * BLK + h, :N])
    out = np.concatenate(pieces, axis=0)
    return out


kernel.last_exec_time_ns = None
